# revision 7
# baseline (speedup 1.0000x reference)
"""GCN layer kernel for Trainium2, 8 NeuronCores.

Computation (see reference): out = relu(segment_sum(vals * (X @ W)[cols], rows))
with X = concat(u_f, v_f) [100000, 128], 1.6M edges.

v2 strategy (fused, collective-free):
  - Associativity: relu((A @ (X@W))) == relu(((A@X) @ W)).  Aggregate raw
    bf16 X rows first, apply the 128x128 weight once per 128-dest block
    AFTER aggregation.  This removes the dense pre-pass, the node_f DRAM
    round-trip, and every collective: each core only needs the full X
    (25.6 MB bf16) in its own DRAM, which the host stages per-core.
  - Destination nodes sharded across 8 cores (12500 rows each, 98 blocks of
    128).  Edges bucketed by (dest block, source window); 4 windows of 25000
    source rows keep gather indices int16.
  - Per (superblock of 8 dest blocks, window) one dma_gather pulls the
    source rows [128 edges x 128 feat] bf16 tiles straight from X in DRAM
    (single-packet descriptor packing).
  - Segment sum via selection matmuls in transposed orientation:
    Sel[e, d] = vals[e] * (d == rows[e]) built in one DVE tensor_scalar
    (in0=iota, scalar1=rows, scalar2=vals, is_equal then mult, all bf16);
    PSUM accumulates accT[f, d] += G[e, f]^T @ Sel[e, d] via
    matmul(lhsT=G, rhs=Sel).
  - accT blocks are copied (ACT, cast bf16) into 4-block groups [128, 512];
    one matmul(lhsT=W16, rhs=Y^T group) applies the weight, ACT relus the
    result to fp32, and it is stored to a transposed output [128, 12500]
    that the host un-transposes for free.

kernel(**inputs) takes full unsharded numpy inputs and returns the full
[100000, 128] float32 output.
"""

import math

import numpy as np

import concourse.tile as tile
from concourse import bacc, mybir
from concourse.bass_utils import run_bass_kernel_spmd

P = 128
N_CORES = 8
N_NODES = 100000
D = 128
DESTS_PER_CORE = N_NODES // N_CORES  # 12500
N_BLOCKS = math.ceil(DESTS_PER_CORE / P)  # 98 blocks of 128 dests (last 84)
N_WINDOWS = 4
WINDOW_ROWS = N_NODES // N_WINDOWS  # 25000 source rows per gather window
SUPER = 8  # dest blocks per gather superblock
GROUP = 4  # dest blocks per weight-matmul group
F32 = mybir.dt.float32
BF16 = mybir.dt.bfloat16
I16 = mybir.dt.int16


def _superblocks():
    return [
        list(range(s, min(s + SUPER, N_BLOCKS))) for s in range(0, N_BLOCKS, SUPER)
    ]


def _groups():
    return [
        list(range(s, min(s + GROUP, N_BLOCKS))) for s in range(0, N_BLOCKS, GROUP)
    ]


def _build_program(cell_sizes, nt_total, idx_cols_total, single_packet=False):
    """Build the SPMD Bass program (identical across cores).

    cell_sizes[b][q]: padded edge count of (dest block b, window q), multiple
    of 128, same for all cores.
    """
    nc = bacc.Bacc(
        "TRN2",
        target_bir_lowering=False,
        debug=False,
        num_swdge_queues=4,
        num_devices=N_CORES,
        dynamic_dma_scratch_size=49152,
    )

    x_in = nc.dram_tensor("x", [N_NODES, D], BF16, kind="ExternalInput")
    w_in = nc.dram_tensor("w", [P, D], F32, kind="ExternalInput")
    rows_in = nc.dram_tensor("rows", [P, nt_total], BF16, kind="ExternalInput")
    vals_in = nc.dram_tensor("vals", [P, nt_total], BF16, kind="ExternalInput")
    idxs_in = nc.dram_tensor("idxs", [P, idx_cols_total], I16, kind="ExternalInput")
    iota_in = nc.dram_tensor("iota", [P, P], BF16, kind="ExternalInput")
    out_t = nc.dram_tensor("out_t", [P, DESTS_PER_CORE], F32, kind="ExternalOutput")

    sblocks = _superblocks()
    max_blk_tiles = max(sum(cs) // P for cs in cell_sizes)
    max_sb_tiles = max(
        sum(cell_sizes[b][q] for b in sb) // P for sb in sblocks for q in range(N_WINDOWS)
    )

    with tile.TileContext(nc) as tc:
        with (
            tc.tile_pool(name="const", bufs=1) as const_pool,
            tc.tile_pool(name="gpool", bufs=6) as g_pool,
            tc.tile_pool(name="onehotpool", bufs=3) as oh_pool,
            tc.tile_pool(name="selpool", bufs=3) as sel_pool,
            tc.tile_pool(name="ytpool", bufs=3) as yt_pool,
            tc.tile_pool(name="outstage", bufs=3) as out_pool,
            tc.tile_pool(name="psum", bufs=2, space="PSUM") as psum_pool,
        ):
            # ---- persistent SBUF state ----
            w_sb = const_pool.tile([P, D], F32, tag="w")
            nc.sync.dma_start(w_sb[:], w_in[:])
            iota_sb = const_pool.tile([P, 1, P], BF16, tag="iota")
            nc.sync.dma_start(iota_sb[:, 0, :], iota_in[:])
            rows_sb = const_pool.tile([P, nt_total], BF16, tag="rows")
            nc.sync.dma_start(rows_sb[:], rows_in[:])
            vals_sb = const_pool.tile([P, nt_total], BF16, tag="vals")
            nc.sync.dma_start(vals_sb[:], vals_in[:])
            idxs_sb = const_pool.tile([P, idx_cols_total], I16, tag="idxs")
            nc.sync.dma_start(idxs_sb[:], idxs_in[:])
            w16_sb = const_pool.tile([P, D], BF16, tag="w16")
            nc.scalar.activation(
                out=w16_sb[:], in_=w_sb[:], func=mybir.ActivationFunctionType.Copy
            )

            # ---- edge phase ----
            groups = _groups()
            grp_of_block = {}
            for gi, g in enumerate(groups):
                for b in g:
                    grp_of_block[b] = gi
            yt_tiles = {}  # group idx -> (tile, width)
            done_blocks_in_grp = {gi: 0 for gi in range(len(groups))}

            gq = 0  # gather queue rotation
            tile_pos = 0  # running tile index into rows/vals (matmul order)
            idx_pos = 0  # running int16 column index into idxs (gather order)
            for sb in sblocks:
                # one gather per (superblock, window)
                g_tiles = {}  # q -> (tile handle, {b: tile offset})
                for q in range(N_WINDOWS):
                    cell_n = sum(cell_sizes[b][q] for b in sb)
                    if cell_n == 0:
                        continue
                    n_tiles = cell_n // P
                    g_sb = g_pool.tile([P, max_sb_tiles, P], BF16, tag="g")
                    nc.gpsimd.dma_gather(
                        g_sb[:, :n_tiles, :],
                        x_in[q * WINDOW_ROWS : (q + 1) * WINDOW_ROWS, :],
                        idxs_sb[:, idx_pos : idx_pos + cell_n // 16],
                        cell_n,
                        cell_n,
                        D,
                        single_packet=single_packet,
                        queue_num=gq,
                    )
                    gq = (gq + 1) % 4
                    idx_pos += cell_n // 16
                    offs = {}
                    off = 0
                    for b in sb:
                        offs[b] = off
                        off += cell_sizes[b][q] // P
                    g_tiles[q] = (g_sb, offs)

                for b in sb:
                    bs = min(P, DESTS_PER_CORE - b * P)
                    acc_t = psum_pool.tile([P, P], F32, tag="accT", bufs=6)
                    block_tiles = sum(cell_sizes[b]) // P
                    # batched selection build: one-hot then vals, whole block
                    t0 = tile_pos
                    nt_b = block_tiles
                    oh = oh_pool.tile([P, max_blk_tiles, P], BF16, tag="oh")
                    nc.vector.tensor_tensor(
                        out=oh[:, :nt_b, :],
                        in0=iota_sb[:].to_broadcast([P, nt_b, P]),
                        in1=rows_sb[:, t0 : t0 + nt_b].to_broadcast([P, nt_b, P]),
                        op=mybir.AluOpType.is_equal,
                    )
                    sel = sel_pool.tile([P, max_blk_tiles, P], BF16, tag="sel")
                    nc.vector.tensor_tensor(
                        out=sel[:, :nt_b, :],
                        in0=oh[:, :nt_b, :],
                        in1=vals_sb[:, t0 : t0 + nt_b].to_broadcast([P, nt_b, P]),
                        op=mybir.AluOpType.mult,
                    )
                    done = 0
                    for q in range(N_WINDOWS):
                        n_tiles = cell_sizes[b][q] // P
                        if n_tiles == 0:
                            continue
                        g_sb, offs = g_tiles[q]
                        for t in range(n_tiles):
                            nc.tensor.matmul(
                                out=acc_t[:],
                                lhsT=g_sb[:, offs[b] + t, :],
                                rhs=sel[:, done, :],
                                start=(done == 0),
                                stop=(done == block_tiles - 1),
                            )
                            done += 1
                            tile_pos += 1
                    # stage Y^T block into its 4-block group buffer (bf16)
                    gi = grp_of_block[b]
                    if gi not in yt_tiles:
                        gwidth = sum(
                            min(P, DESTS_PER_CORE - bb * P) for bb in groups[gi]
                        )
                        yt_sb = yt_pool.tile([P, GROUP * P], BF16, tag="yt")
                        yt_tiles[gi] = (yt_sb, gwidth)
                    yt_sb, gwidth = yt_tiles[gi]
                    slot = b - groups[gi][0]
                    nc.scalar.activation(
                        out=yt_sb[:, slot * P : slot * P + bs],
                        in_=acc_t[:, :bs],
                        func=mybir.ActivationFunctionType.Copy,
                    )
                    done_blocks_in_grp[gi] += 1
                    # group complete -> weight matmul + relu + store
                    if done_blocks_in_grp[gi] == len(groups[gi]):
                        o_ps = psum_pool.tile([P, GROUP * P], F32, tag="ops", bufs=2)
                        nc.tensor.matmul(
                            out=o_ps[:, :gwidth],
                            lhsT=w16_sb[:],
                            rhs=yt_sb[:, :gwidth],
                            start=True,
                            stop=True,
                        )
                        ostage = out_pool.tile([P, GROUP * P], F32, tag="ostage")
                        nc.scalar.activation(
                            out=ostage[:, :gwidth],
                            in_=o_ps[:, :gwidth],
                            func=mybir.ActivationFunctionType.Relu,
                        )
                        goff = groups[gi][0] * P
                        nc.sync.dma_start(
                            out_t[:, goff : goff + gwidth], ostage[:, :gwidth]
                        )
                        del yt_tiles[gi]

    nc.compile()
    return nc


_CACHE = {}


def _prepare(u_f, v_f, adj_rows, adj_cols, adj_vals):
    """Host-side sharding: bucket edges by (core, superblock, window, block),
    pad (block, window) subcells to multiples of 128 (uniform across cores),
    and lay out per-core rows/vals/idx arrays in the SBUF tile layouts.

    Gather (idxs) order: superblock -> window -> block -> edges.
    Matmul (rows/vals) order: superblock -> block -> window -> edges.
    """
    import ml_dtypes

    rows = np.asarray(adj_rows, dtype=np.int64)
    cols = np.asarray(adj_cols, dtype=np.int64)
    vals = np.asarray(adj_vals, dtype=np.float32)

    core_of = rows // DESTS_PER_CORE
    blk_of = (rows % DESTS_PER_CORE) // P
    win_of = cols // WINDOW_ROWS
    win_idx = cols % WINDOW_ROWS

    key = (core_of * N_BLOCKS + blk_of) * N_WINDOWS + win_of
    order = np.argsort(key, kind="stable")
    rows_s = rows[order]
    widx_s = win_idx[order]
    vals_s = vals[order]

    n_cells_total = N_CORES * N_BLOCKS * N_WINDOWS
    counts = np.bincount(key[order], minlength=n_cells_total).reshape(
        N_CORES, N_BLOCKS, N_WINDOWS
    )
    starts = np.zeros(n_cells_total + 1, dtype=np.int64)
    np.cumsum(counts.reshape(-1), out=starts[1:])

    max_counts = counts.max(axis=0)  # [N_BLOCKS, N_WINDOWS]
    cell_sizes = (np.ceil(max_counts / P).astype(np.int64) * P).tolist()
    for b in range(N_BLOCKS):
        if sum(cell_sizes[b]) == 0:
            cell_sizes[b][0] = P  # keep PSUM written for empty blocks

    total_padded = sum(sum(cs) for cs in cell_sizes)
    nt_total = total_padded // P
    idx_cols_total = total_padded // 16
    sblocks = _superblocks()

    per_core = []
    for c in range(N_CORES):
        rows_t = np.zeros((P, nt_total), ml_dtypes.bfloat16)
        vals_t = np.zeros((P, nt_total), ml_dtypes.bfloat16)
        idxs_t = np.zeros((P, idx_cols_total), np.int16)
        tile_pos = 0
        idx_pos = 0

        def cell_edges(b, q):
            ci = (c * N_BLOCKS + b) * N_WINDOWS + q
            s = starts[ci]
            e = s + counts[c, b, q]
            # sort by source index for HBM row locality in the gather
            o = np.argsort(widx_s[s:e], kind="stable")
            return rows_s[s:e][o], widx_s[s:e][o], vals_s[s:e][o]

        for sb in sblocks:
            # gather (idxs) order: window -> block
            for q in range(N_WINDOWS):
                for b in sb:
                    cell_n = cell_sizes[b][q]
                    if cell_n == 0:
                        continue
                    _, cc, _ = cell_edges(b, q)
                    ii = np.zeros(cell_n, np.int16)
                    ii[: len(cc)] = cc.astype(np.int16)
                    wrapped = ii.reshape(cell_n // 16, 16).T
                    for g in range(8):
                        idxs_t[
                            16 * g : 16 * (g + 1), idx_pos : idx_pos + cell_n // 16
                        ] = wrapped
                    idx_pos += cell_n // 16
            # matmul (rows/vals) order: block -> window
            for b in sb:
                for q in range(N_WINDOWS):
                    cell_n = cell_sizes[b][q]
                    if cell_n == 0:
                        continue
                    rr, _, vv = cell_edges(b, q)
                    r = np.zeros(cell_n, np.float32)
                    v = np.zeros(cell_n, np.float32)
                    r[: len(rr)] = (rr - c * DESTS_PER_CORE - b * P).astype(np.float32)
                    v[: len(vv)] = vv
                    nt = cell_n // P
                    rows_t[:, tile_pos : tile_pos + nt] = (
                        r.reshape(nt, P).T.astype(ml_dtypes.bfloat16)
                    )
                    vals_t[:, tile_pos : tile_pos + nt] = (
                        v.reshape(nt, P).T.astype(ml_dtypes.bfloat16)
                    )
                    tile_pos += nt
        per_core.append((rows_t, vals_t, idxs_t))

    x_full = np.concatenate(
        [np.asarray(u_f, np.float32), np.asarray(v_f, np.float32)], axis=0
    ).astype(ml_dtypes.bfloat16)
    iota = np.broadcast_to(
        np.arange(P).astype(ml_dtypes.bfloat16), (P, P)
    ).copy()
    in_maps = []
    for c in range(N_CORES):
        rows_t, vals_t, idxs_t = per_core[c]
        in_maps.append(
            {
                "x": x_full,
                "w": None,  # filled by caller
                "rows": rows_t,
                "vals": vals_t,
                "idxs": idxs_t,
                "iota": iota,
            }
        )
    return cell_sizes, nt_total, idx_cols_total, in_maps


def kernel(u_f, v_f, adj_rows, adj_cols, adj_vals, weight):
    w = np.asarray(weight, np.float32)
    cell_sizes, nt_total, idx_cols_total, in_maps = _prepare(
        u_f, v_f, adj_rows, adj_cols, adj_vals
    )
    for m in in_maps:
        m["w"] = w

    cache_key = (nt_total, idx_cols_total, tuple(tuple(cs) for cs in cell_sizes))
    if cache_key not in _CACHE:
        _CACHE.clear()
        _CACHE[cache_key] = _build_program(cell_sizes, nt_total, idx_cols_total)
    nc = _CACHE[cache_key]

    # The axon-tunneled device occasionally reports a transient
    # NRT_EXEC_UNIT_UNRECOVERABLE from a previous crashed run; a retry runs
    # on the freshly-reset device.
    last_err = None
    for _ in range(4):
        try:
            res = run_bass_kernel_spmd(nc, in_maps, core_ids=list(range(N_CORES)))
            break
        except Exception as e:  # noqa: BLE001
            last_err = e
    else:
        raise last_err
    return np.concatenate(
        [
            np.ascontiguousarray(res.results[c]["out_t"].T)
            for c in range(N_CORES)
        ],
        axis=0,
    )


# revision 9
# speedup vs baseline: 1.9194x; 1.9194x over previous
"""GCN layer kernel for Trainium2, 8 NeuronCores.

Computation (see reference): out = relu(segment_sum(vals * (X @ W)[cols], rows))
with X = concat(u_f, v_f) [100000, 128], 1.6M edges.

v2 strategy (fused, collective-free):
  - Associativity: relu((A @ (X@W))) == relu(((A@X) @ W)).  Aggregate raw
    bf16 X rows first, apply the 128x128 weight once per 128-dest block
    AFTER aggregation.  This removes the dense pre-pass, the node_f DRAM
    round-trip, and every collective: each core only needs the full X
    (25.6 MB bf16) in its own DRAM, which the host stages per-core.
  - Destination nodes sharded across 8 cores (12500 rows each, 98 blocks of
    128).  Edges bucketed by (dest block, source window); 4 windows of 25000
    source rows keep gather indices int16.
  - Per (superblock of 8 dest blocks, window) one dma_gather pulls the
    source rows [128 edges x 128 feat] bf16 tiles straight from X in DRAM
    (single-packet descriptor packing).
  - Segment sum via selection matmuls in transposed orientation:
    Sel[e, d] = vals[e] * (d == rows[e]) built in one DVE tensor_scalar
    (in0=iota, scalar1=rows, scalar2=vals, is_equal then mult, all bf16);
    PSUM accumulates accT[f, d] += G[e, f]^T @ Sel[e, d] via
    matmul(lhsT=G, rhs=Sel).
  - accT blocks are copied (ACT, cast bf16) into 4-block groups [128, 512];
    one matmul(lhsT=W16, rhs=Y^T group) applies the weight, ACT relus the
    result to fp32, and it is stored to a transposed output [128, 12500]
    that the host un-transposes for free.

kernel(**inputs) takes full unsharded numpy inputs and returns the full
[100000, 128] float32 output.
"""

import math

import numpy as np

import concourse.tile as tile
from concourse import bacc, mybir
from concourse.bass_utils import run_bass_kernel_spmd

P = 128
N_CORES = 8
N_NODES = 100000
D = 128
DESTS_PER_CORE = N_NODES // N_CORES  # 12500
N_BLOCKS = math.ceil(DESTS_PER_CORE / P)  # 98 blocks of 128 dests (last 84)
N_WINDOWS = 4
WINDOW_ROWS = N_NODES // N_WINDOWS  # 25000 source rows per gather window
SUPER = 8  # dest blocks per gather superblock
GROUP = 4  # dest blocks per weight-matmul group
CHUNK_EDGES = 1024  # max descriptors per dma_gather (~ring capacity)
F32 = mybir.dt.float32
BF16 = mybir.dt.bfloat16
I16 = mybir.dt.int16


def _superblocks():
    return [
        list(range(s, min(s + SUPER, N_BLOCKS))) for s in range(0, N_BLOCKS, SUPER)
    ]


def _groups():
    return [
        list(range(s, min(s + GROUP, N_BLOCKS))) for s in range(0, N_BLOCKS, GROUP)
    ]


def _build_program(cell_sizes, nt_total, idx_cols_total, single_packet=False):
    """Build the SPMD Bass program (identical across cores).

    cell_sizes[b][q]: padded edge count of (dest block b, window q), multiple
    of 128, same for all cores.
    """
    nc = bacc.Bacc(
        "TRN2",
        target_bir_lowering=False,
        debug=False,
        num_swdge_queues=4,
        num_devices=N_CORES,
        dynamic_dma_scratch_size=49152,
    )

    x_in = nc.dram_tensor("x", [N_NODES, D], BF16, kind="ExternalInput")
    w_in = nc.dram_tensor("w", [P, D], F32, kind="ExternalInput")
    rows_in = nc.dram_tensor("rows", [P, nt_total], BF16, kind="ExternalInput")
    vals_in = nc.dram_tensor("vals", [P, nt_total], BF16, kind="ExternalInput")
    idxs_in = nc.dram_tensor("idxs", [P, idx_cols_total], I16, kind="ExternalInput")
    iota_in = nc.dram_tensor("iota", [P, P], BF16, kind="ExternalInput")
    out_t = nc.dram_tensor("out_t", [P, DESTS_PER_CORE], F32, kind="ExternalOutput")

    sblocks = _superblocks()
    max_blk_tiles = max(sum(cs) // P for cs in cell_sizes)
    max_sb_tiles = max(
        sum(cell_sizes[b][q] for b in sb) // P for sb in sblocks for q in range(N_WINDOWS)
    )

    with tile.TileContext(nc) as tc:
        with (
            tc.tile_pool(name="const", bufs=1) as const_pool,
            tc.tile_pool(name="gpool", bufs=6) as g_pool,
            tc.tile_pool(name="onehotpool", bufs=3) as oh_pool,
            tc.tile_pool(name="selpool", bufs=3) as sel_pool,
            tc.tile_pool(name="ytpool", bufs=3) as yt_pool,
            tc.tile_pool(name="outstage", bufs=3) as out_pool,
            tc.tile_pool(name="psum", bufs=2, space="PSUM") as psum_pool,
        ):
            # ---- persistent SBUF state ----
            w_sb = const_pool.tile([P, D], F32, tag="w")
            nc.sync.dma_start(w_sb[:], w_in[:])
            iota_sb = const_pool.tile([P, 1, P], BF16, tag="iota")
            nc.sync.dma_start(iota_sb[:, 0, :], iota_in[:])
            rows_sb = const_pool.tile([P, nt_total], BF16, tag="rows")
            nc.sync.dma_start(rows_sb[:], rows_in[:])
            vals_sb = const_pool.tile([P, nt_total], BF16, tag="vals")
            nc.sync.dma_start(vals_sb[:], vals_in[:])
            idxs_sb = const_pool.tile([P, idx_cols_total], I16, tag="idxs")
            nc.sync.dma_start(idxs_sb[:], idxs_in[:])
            w16_sb = const_pool.tile([P, D], BF16, tag="w16")
            nc.scalar.activation(
                out=w16_sb[:], in_=w_sb[:], func=mybir.ActivationFunctionType.Copy
            )

            # ---- edge phase ----
            groups = _groups()
            grp_of_block = {}
            for gi, g in enumerate(groups):
                for b in g:
                    grp_of_block[b] = gi
            yt_tiles = {}  # group idx -> (tile, width)
            done_blocks_in_grp = {gi: 0 for gi in range(len(groups))}

            gq = 0  # gather queue rotation (round-robin per sub-gather)
            tile_pos = 0  # running tile index into rows/vals (matmul order)
            idx_pos = 0  # running int16 column index into idxs (gather order)
            for sb in sblocks:
                # Per (superblock, window): gather into one buffer, but split
                # into sub-gathers of <= CHUNK_EDGES descriptors so the Q7
                # never blocks long on descriptor-ring space and the four
                # SWDGE queues drain concurrently.
                g_tiles = {}  # q -> (tile handle, {b: tile offset})
                for q in range(N_WINDOWS):
                    cell_n = sum(cell_sizes[b][q] for b in sb)
                    if cell_n == 0:
                        continue
                    g_sb = g_pool.tile([P, max_sb_tiles, P], BF16, tag="g")
                    done_e = 0
                    while done_e < cell_n:
                        chunk = min(CHUNK_EDGES, cell_n - done_e)
                        t_off = done_e // P
                        nc.gpsimd.dma_gather(
                            g_sb[:, t_off : t_off + chunk // P, :],
                            x_in[q * WINDOW_ROWS : (q + 1) * WINDOW_ROWS, :],
                            idxs_sb[
                                :, idx_pos + done_e // 16 : idx_pos + (done_e + chunk) // 16
                            ],
                            chunk,
                            chunk,
                            D,
                            single_packet=single_packet,
                            queue_num=gq,
                        )
                        gq = (gq + 1) % 4
                        done_e += chunk
                    idx_pos += cell_n // 16
                    offs = {}
                    off = 0
                    for b in sb:
                        offs[b] = off
                        off += cell_sizes[b][q] // P
                    g_tiles[q] = (g_sb, offs)

                for b in sb:
                    bs = min(P, DESTS_PER_CORE - b * P)
                    acc_t = psum_pool.tile([P, P], F32, tag="accT", bufs=6)
                    block_tiles = sum(cell_sizes[b]) // P
                    # batched selection build: one-hot then vals, whole block
                    t0 = tile_pos
                    nt_b = block_tiles
                    oh = oh_pool.tile([P, max_blk_tiles, P], BF16, tag="oh")
                    nc.vector.tensor_tensor(
                        out=oh[:, :nt_b, :],
                        in0=iota_sb[:].to_broadcast([P, nt_b, P]),
                        in1=rows_sb[:, t0 : t0 + nt_b].to_broadcast([P, nt_b, P]),
                        op=mybir.AluOpType.is_equal,
                    )
                    sel = sel_pool.tile([P, max_blk_tiles, P], BF16, tag="sel")
                    nc.vector.tensor_tensor(
                        out=sel[:, :nt_b, :],
                        in0=oh[:, :nt_b, :],
                        in1=vals_sb[:, t0 : t0 + nt_b].to_broadcast([P, nt_b, P]),
                        op=mybir.AluOpType.mult,
                    )
                    done = 0
                    for q in range(N_WINDOWS):
                        n_tiles = cell_sizes[b][q] // P
                        if n_tiles == 0:
                            continue
                        g_sb, offs = g_tiles[q]
                        for t in range(n_tiles):
                            nc.tensor.matmul(
                                out=acc_t[:],
                                lhsT=g_sb[:, offs[b] + t, :],
                                rhs=sel[:, done, :],
                                start=(done == 0),
                                stop=(done == block_tiles - 1),
                            )
                            done += 1
                            tile_pos += 1
                    # stage Y^T block into its 4-block group buffer (bf16)
                    gi = grp_of_block[b]
                    if gi not in yt_tiles:
                        gwidth = sum(
                            min(P, DESTS_PER_CORE - bb * P) for bb in groups[gi]
                        )
                        yt_sb = yt_pool.tile([P, GROUP * P], BF16, tag="yt")
                        yt_tiles[gi] = (yt_sb, gwidth)
                    yt_sb, gwidth = yt_tiles[gi]
                    slot = b - groups[gi][0]
                    nc.scalar.activation(
                        out=yt_sb[:, slot * P : slot * P + bs],
                        in_=acc_t[:, :bs],
                        func=mybir.ActivationFunctionType.Copy,
                    )
                    done_blocks_in_grp[gi] += 1
                    # group complete -> weight matmul + relu + store
                    if done_blocks_in_grp[gi] == len(groups[gi]):
                        o_ps = psum_pool.tile([P, GROUP * P], F32, tag="ops", bufs=2)
                        nc.tensor.matmul(
                            out=o_ps[:, :gwidth],
                            lhsT=w16_sb[:],
                            rhs=yt_sb[:, :gwidth],
                            start=True,
                            stop=True,
                        )
                        ostage = out_pool.tile([P, GROUP * P], F32, tag="ostage")
                        nc.scalar.activation(
                            out=ostage[:, :gwidth],
                            in_=o_ps[:, :gwidth],
                            func=mybir.ActivationFunctionType.Relu,
                        )
                        goff = groups[gi][0] * P
                        nc.sync.dma_start(
                            out_t[:, goff : goff + gwidth], ostage[:, :gwidth]
                        )
                        del yt_tiles[gi]

    nc.compile()
    return nc


_CACHE = {}


def _prepare(u_f, v_f, adj_rows, adj_cols, adj_vals):
    """Host-side sharding: bucket edges by (core, superblock, window, block),
    pad (block, window) subcells to multiples of 128 (uniform across cores),
    and lay out per-core rows/vals/idx arrays in the SBUF tile layouts.

    Gather (idxs) order: superblock -> window -> block -> edges.
    Matmul (rows/vals) order: superblock -> block -> window -> edges.
    """
    import ml_dtypes

    rows = np.asarray(adj_rows, dtype=np.int64)
    cols = np.asarray(adj_cols, dtype=np.int64)
    vals = np.asarray(adj_vals, dtype=np.float32)

    core_of = rows // DESTS_PER_CORE
    blk_of = (rows % DESTS_PER_CORE) // P
    win_of = cols // WINDOW_ROWS
    win_idx = cols % WINDOW_ROWS

    key = (core_of * N_BLOCKS + blk_of) * N_WINDOWS + win_of
    order = np.argsort(key, kind="stable")
    rows_s = rows[order]
    widx_s = win_idx[order]
    vals_s = vals[order]

    n_cells_total = N_CORES * N_BLOCKS * N_WINDOWS
    counts = np.bincount(key[order], minlength=n_cells_total).reshape(
        N_CORES, N_BLOCKS, N_WINDOWS
    )
    starts = np.zeros(n_cells_total + 1, dtype=np.int64)
    np.cumsum(counts.reshape(-1), out=starts[1:])

    max_counts = counts.max(axis=0)  # [N_BLOCKS, N_WINDOWS]
    cell_sizes = (np.ceil(max_counts / P).astype(np.int64) * P).tolist()
    for b in range(N_BLOCKS):
        if sum(cell_sizes[b]) == 0:
            cell_sizes[b][0] = P  # keep PSUM written for empty blocks

    total_padded = sum(sum(cs) for cs in cell_sizes)
    nt_total = total_padded // P
    idx_cols_total = total_padded // 16
    sblocks = _superblocks()

    per_core = []
    for c in range(N_CORES):
        rows_t = np.zeros((P, nt_total), ml_dtypes.bfloat16)
        vals_t = np.zeros((P, nt_total), ml_dtypes.bfloat16)
        idxs_t = np.zeros((P, idx_cols_total), np.int16)
        tile_pos = 0
        idx_pos = 0

        def cell_edges(b, q):
            ci = (c * N_BLOCKS + b) * N_WINDOWS + q
            s = starts[ci]
            e = s + counts[c, b, q]
            # sort by source index for HBM row locality in the gather
            o = np.argsort(widx_s[s:e], kind="stable")
            return rows_s[s:e][o], widx_s[s:e][o], vals_s[s:e][o]

        for sb in sblocks:
            # gather (idxs) order: window -> block
            for q in range(N_WINDOWS):
                for b in sb:
                    cell_n = cell_sizes[b][q]
                    if cell_n == 0:
                        continue
                    _, cc, _ = cell_edges(b, q)
                    ii = np.zeros(cell_n, np.int16)
                    ii[: len(cc)] = cc.astype(np.int16)
                    wrapped = ii.reshape(cell_n // 16, 16).T
                    for g in range(8):
                        idxs_t[
                            16 * g : 16 * (g + 1), idx_pos : idx_pos + cell_n // 16
                        ] = wrapped
                    idx_pos += cell_n // 16
            # matmul (rows/vals) order: block -> window
            for b in sb:
                for q in range(N_WINDOWS):
                    cell_n = cell_sizes[b][q]
                    if cell_n == 0:
                        continue
                    rr, _, vv = cell_edges(b, q)
                    r = np.zeros(cell_n, np.float32)
                    v = np.zeros(cell_n, np.float32)
                    r[: len(rr)] = (rr - c * DESTS_PER_CORE - b * P).astype(np.float32)
                    v[: len(vv)] = vv
                    nt = cell_n // P
                    rows_t[:, tile_pos : tile_pos + nt] = (
                        r.reshape(nt, P).T.astype(ml_dtypes.bfloat16)
                    )
                    vals_t[:, tile_pos : tile_pos + nt] = (
                        v.reshape(nt, P).T.astype(ml_dtypes.bfloat16)
                    )
                    tile_pos += nt
        per_core.append((rows_t, vals_t, idxs_t))

    x_full = np.concatenate(
        [np.asarray(u_f, np.float32), np.asarray(v_f, np.float32)], axis=0
    ).astype(ml_dtypes.bfloat16)
    iota = np.broadcast_to(
        np.arange(P).astype(ml_dtypes.bfloat16), (P, P)
    ).copy()
    in_maps = []
    for c in range(N_CORES):
        rows_t, vals_t, idxs_t = per_core[c]
        in_maps.append(
            {
                "x": x_full,
                "w": None,  # filled by caller
                "rows": rows_t,
                "vals": vals_t,
                "idxs": idxs_t,
                "iota": iota,
            }
        )
    return cell_sizes, nt_total, idx_cols_total, in_maps


def kernel(u_f, v_f, adj_rows, adj_cols, adj_vals, weight):
    w = np.asarray(weight, np.float32)
    cell_sizes, nt_total, idx_cols_total, in_maps = _prepare(
        u_f, v_f, adj_rows, adj_cols, adj_vals
    )
    for m in in_maps:
        m["w"] = w

    cache_key = (nt_total, idx_cols_total, tuple(tuple(cs) for cs in cell_sizes))
    if cache_key not in _CACHE:
        _CACHE.clear()
        _CACHE[cache_key] = _build_program(cell_sizes, nt_total, idx_cols_total)
    nc = _CACHE[cache_key]

    # The axon-tunneled device occasionally reports a transient
    # NRT_EXEC_UNIT_UNRECOVERABLE from a previous crashed run; a retry runs
    # on the freshly-reset device.
    last_err = None
    for _ in range(4):
        try:
            res = run_bass_kernel_spmd(nc, in_maps, core_ids=list(range(N_CORES)))
            break
        except Exception as e:  # noqa: BLE001
            last_err = e
    else:
        raise last_err
    return np.concatenate(
        [
            np.ascontiguousarray(res.results[c]["out_t"].T)
            for c in range(N_CORES)
        ],
        axis=0,
    )
